# revision 9
# baseline (speedup 1.0000x reference)
"""Multi-head attention with KV cache, tensor-parallel over 8 TRN2 NeuronCores.

Sharding: heads are split 2-per-core (H=16, 8 cores). Each core computes its
heads' Q/K/V projections (output columns), attention, and a partial O
projection (input rows); a ReduceScatter sums the partials and hands each core
one token shard of the final output. The host reassembles full outputs.

All matmuls run as float32r (full-rate PE, ~1.5e-4 relative error).
"""

import numpy as np

import concourse.bass as bass
import concourse.mybir as mybir
from concourse.tile import TileContext
from concourse.bass_utils import run_bass_kernel_spmd
from concourse.tile_rust import add_dep_helper

F32 = mybir.dt.float32
F32R = mybir.dt.float32r
EXP = mybir.ActivationFunctionType.Exp

B, S, E = 2, 2048, 2048
H, DH, PAST = 16, 128, 2048
NCORES = 8
HL = H // NCORES          # heads per core (2)
DL = HL * DH              # per-core head dims (256)
T = B * S                 # total tokens (4096)
CH = 256                  # projection token-chunk
NCHUNK = T // CH          # 16
SCW = 512                 # attention query-chunk width
NSC = S // SCW            # 4 query chunks per batch
KV = PAST + S             # 4096
NT = KV // 128            # 32 key tiles per (batch, head)
NT_PAST = PAST // 128     # 16
NEG = -1.0e30
SCALE = float(1.0 / np.sqrt(np.float32(DH)))

_CACHE = {}


def _fix_walrus_wait_limit(nc, max_waits=1):
    """This walrus build rejects >1 sync-wait command per instruction; move
    excess waits onto same-engine nops inserted right before the offender."""
    end_bb = nc.cur_bb.bb
    for fn in nc.m.functions:
        for bb in fn.blocks:
            insts = bb.instructions
            i = 0
            while i < len(insts):
                inst = insts[i]
                si = inst.sync_info
                if si is None or not si.on_wait or len(si.on_wait) <= max_waits:
                    i += 1
                    continue
                waits = list(si.on_wait)
                keep = waits[-max_waits:]
                extra = waits[:-max_waits]
                si.on_wait.clear()
                si.on_wait.extend(keep)
                eng = nc.engines[inst.engine]
                nops = []
                for j in range(0, len(extra), max_waits):
                    nop_inst = eng.nop(nofuse=True, hint="waitsplit").ins
                    nsi = nop_inst.sync_info
                    if nsi is None:
                        nop_inst.sync_info = mybir.SyncInfo(on_wait=[], on_update=[])
                        nsi = nop_inst.sync_info
                    nsi.on_wait.extend(extra[j:j + max_waits])
                    end_bb.instructions.remove(nop_inst)
                    nops.append(nop_inst)
                for j, n in enumerate(nops):
                    insts.insert(i + j, n)
                i += len(nops) + 1


def _build():
    nc = bass.Bass(num_devices=NCORES)

    xT = nc.declare_dram_parameter("xT", [E, T], F32, isOutput=False)
    wqT = nc.declare_dram_parameter("wqT", [E, DL], F32, isOutput=False)
    wkT = nc.declare_dram_parameter("wkT", [E, DL], F32, isOutput=False)
    wvT = nc.declare_dram_parameter("wvT", [E, DL], F32, isOutput=False)
    woT = nc.declare_dram_parameter("woT", [DL, E], F32, isOutput=False)
    bq2 = nc.declare_dram_parameter("bq2", [HL, 128], F32, isOutput=False)
    bk2 = nc.declare_dram_parameter("bk2", [HL, 128], F32, isOutput=False)
    bv_rep = nc.declare_dram_parameter("bv_rep", [128, DL], F32, isOutput=False)
    bo_rep = nc.declare_dram_parameter("bo_rep", [128, E], F32, isOutput=False)
    pkT = nc.declare_dram_parameter("pkT", [B, HL, 128, PAST], F32, isOutput=False)
    pv = nc.declare_dram_parameter("pv", [B, HL, PAST, 128], F32, isOutput=False)
    masks = nc.declare_dram_parameter("masks", [4, 128, SCW], F32, isOutput=False)
    ones128 = nc.declare_dram_parameter("ones128", [128], F32, isOutput=False)

    kT_new = nc.declare_dram_parameter("kT_new", [HL, 128, T], F32, isOutput=True)
    v_new = nc.declare_dram_parameter("v_new", [T, DL], F32, isOutput=True)
    o_shard = nc.declare_dram_parameter("o_shard", [T // NCORES, E], F32, isOutput=True)

    o_partial = nc.dram_tensor("o_partial", [T, E], F32)
    rs_out = nc.dram_tensor("rs_out", [T // NCORES, E], F32)

    NKT = E // 128  # 16 contraction tiles for the projections
    kv_writers = {}  # (kind, b, l) or (kind, b) -> [instructions]

    with TileContext(nc) as tc:
        with (
            tc.tile_pool(name="const", bufs=1) as constp,
            tc.tile_pool(name="big", bufs=1) as bigp,
        ):
            # ---- small constants ----
            bq_sb = constp.tile([128, HL], F32, tag="bq")
            bk_sb = constp.tile([128, HL], F32, tag="bk")
            nc.sync.dma_start(out=bq_sb[:], in_=bq2[:].rearrange("l p -> p l"))
            nc.sync.dma_start(out=bk_sb[:], in_=bk2[:].rearrange("l p -> p l"))
            bv_sb = constp.tile([128, DL], F32, tag="bv")
            bo_sb = constp.tile([128, E], F32, tag="bo")
            nc.sync.dma_start(out=bv_sb[:], in_=bv_rep[:])
            nc.sync.dma_start(out=bo_sb[:], in_=bo_rep[:])
            mask_sb = constp.tile([128, 4 * SCW], F32, tag="mask")
            nc.sync.dma_start(out=mask_sb[:].rearrange("p (m f) -> p m f", m=4),
                              in_=masks[:].rearrange("m p f -> p m f"))
            ones_t = constp.tile([128, 1], F32R, tag="onest")
            ones_k = constp.tile([1, 128], F32R, tag="onesk")
            nc.sync.dma_start(out=ones_t[:], in_=ones128[:].unsqueeze(1).bitcast(F32R))
            nc.sync.dma_start(out=ones_k[:], in_=ones128[:].unsqueeze(0).bitcast(F32R))

            # ---- persistent activations (SBUF resident across phases) ----
            qT_sb = [bigp.tile([128, T], F32R, name=f"qT{l}", tag=f"qT{l}")
                     for l in range(HL)]
            ctxT = [[bigp.tile([128, S], F32R, name=f"cx{b}{l}", tag=f"cx{b}{l}")
                     for l in range(HL)] for b in range(B)]

            # ---- phase 1: projections ----
            with (
                tc.tile_pool(name="wp", bufs=1) as wp,
                tc.tile_pool(name="xt", bufs=2) as xtp,
                tc.tile_pool(name="pjps", bufs=2, space="PSUM") as pjps,
                tc.tile_pool(name="ev", bufs=3) as evp,
            ):
                wq = wp.tile([128, NKT * DL], F32R, tag="wq")  # [p, kt*DL + m]
                wk = wp.tile([128, NKT * DL], F32R, tag="wk")
                wv = wp.tile([128, NKT * DL], F32R, tag="wv")
                for wt, wsrc in ((wq, wqT), (wk, wkT), (wv, wvT)):
                    nc.sync.dma_start(
                        out=wt[:].rearrange("p (a m) -> p a m", a=NKT),
                        in_=wsrc[:].rearrange("(a p) m -> p a m", p=128).bitcast(F32R))
                for ci in range(NCHUNK):
                    t0 = ci * CH
                    b = t0 // S
                    xt = xtp.tile([128, NKT * CH], F32R)  # [p, kt*CH + t]
                    nc.sync.dma_start(
                        out=xt[:].rearrange("p (a t) -> p a t", a=NKT),
                        in_=xT[:, t0:t0 + CH].rearrange("(a p) t -> p a t", p=128).bitcast(F32R),
                    )
                    # q/k projections (transposed layout: [dh, tok])
                    for l in range(HL):
                        pqk = pjps.tile([128, 2 * CH], F32, tag="pqk")
                        # q group fully, then k group: a start=True clears the
                        # whole bank's has_written bits, so groups sharing a
                        # bank must not interleave
                        for kt in range(NKT):
                            st, sp = (kt == 0), (kt == NKT - 1)
                            nc.tensor.matmul(pqk[:, 0:CH],
                                             wq[:, kt * DL + l * 128: kt * DL + (l + 1) * 128],
                                             xt[:, kt * CH:(kt + 1) * CH], start=st, stop=sp)
                        for kt in range(NKT):
                            st, sp = (kt == 0), (kt == NKT - 1)
                            nc.tensor.matmul(pqk[:, CH:2 * CH],
                                             wk[:, kt * DL + l * 128: kt * DL + (l + 1) * 128],
                                             xt[:, kt * CH:(kt + 1) * CH], start=st, stop=sp)
                        nc.vector.tensor_scalar_add(qT_sb[l][:, t0:t0 + CH], pqk[:, 0:CH],
                                                    bq_sb[:, l:l + 1])
                        kstg = evp.tile([128, CH], F32R, tag="kstg")
                        nc.vector.tensor_scalar_add(kstg[:], pqk[:, CH:2 * CH],
                                                    bk_sb[:, l:l + 1])
                        kw = nc.sync.dma_start(out=kT_new[l, :, t0:t0 + CH],
                                               in_=kstg[:].bitcast(F32))
                        kv_writers.setdefault(("k", b, l), []).append(kw.ins)
                    # v projection (natural layout: [tok, dh])
                    pv_ps = pjps.tile([128, 2 * DL], F32, tag="pv")
                    for mt in range(CH // 128):
                        for kt in range(NKT):
                            nc.tensor.matmul(pv_ps[:, mt * DL:(mt + 1) * DL],
                                             xt[:, kt * CH + mt * 128: kt * CH + (mt + 1) * 128],
                                             wv[:, kt * DL:(kt + 1) * DL],
                                             start=(kt == 0), stop=(kt == NKT - 1))
                    for mt in range(CH // 128):
                        g = ci * (CH // 128) + mt          # global token tile
                        vf = evp.tile([128, DL], F32, tag="vf")
                        nc.vector.tensor_add(vf[:], pv_ps[:, mt * DL:(mt + 1) * DL], bv_sb[:])
                        vw = nc.sync.dma_start(out=v_new[g * 128:(g + 1) * 128, :], in_=vf[:])
                        kv_writers.setdefault(("v", b), []).append(vw.ins)

            # ---- phase 2: attention ----
            with (
                tc.tile_pool(name="kv", bufs=2) as kvp,
                tc.tile_pool(name="scps", bufs=2, space="PSUM") as scps,
                tc.tile_pool(name="expp", bufs=3) as expp,
                tc.tile_pool(name="ctxps", bufs=2, space="PSUM") as ctxps,
                tc.tile_pool(name="sumps", bufs=1, space="PSUM") as sumps,
                tc.tile_pool(name="repps", bufs=1, space="PSUM") as repps,
                tc.tile_pool(name="small", bufs=2) as smallp,
            ):
                for b in range(B):
                    for l in range(HL):
                        # full K^T and V for this (batch, head): past ++ new
                        katt = kvp.tile([128, NT * 128], F32R, tag="katt")
                        nc.sync.dma_start(out=katt[:, 0:PAST], in_=pkT[b, l].bitcast(F32R))
                        krd = nc.sync.dma_start(out=katt[:, PAST:KV],
                                                in_=kT_new[l, :, b * S:(b + 1) * S].bitcast(F32R))
                        for w in kv_writers.get(("k", b, l), []):
                            add_dep_helper(krd.ins, w, sync=True, reason="kT_new RAW")
                        vatt = kvp.tile([128, NT * 128], F32R, tag="vatt")
                        nc.sync.dma_start(
                            out=vatt[:, 0:PAST].rearrange("p (a d) -> p a d", a=NT_PAST),
                            in_=pv[b, l].rearrange("(a p) d -> p a d", p=128).bitcast(F32R))
                        vrd = nc.sync.dma_start(
                            out=vatt[:, PAST:KV].rearrange("p (a d) -> p a d", a=NT_PAST),
                            in_=v_new[b * S:(b + 1) * S, l * DH:(l + 1) * DH]
                                .rearrange("(a p) d -> p a d", p=128).bitcast(F32R))
                        for w in kv_writers.get(("v", b), []):
                            add_dep_helper(vrd.ins, w, sync=True, reason="v_new RAW")
                        for sc in range(NSC):
                            qs = qT_sb[l][:, b * S + sc * SCW: b * S + (sc + 1) * SCW]
                            ctx_ps = ctxps.tile([128, SCW], F32)
                            sum_ps = sumps.tile([1, SCW], F32)
                            n_vis = NT_PAST + 4 * sc + 4   # visible key tiles
                            pairs = [(i, i + 1) for i in range(0, n_vis, 2)]
                            for pi, pair in enumerate(pairs):
                                sc_ps = scps.tile([128, 2 * SCW], F32)
                                for hf, ti in enumerate(pair):
                                    nc.tensor.matmul(sc_ps[:, hf * SCW:(hf + 1) * SCW],
                                                     katt[:, ti * 128:(ti + 1) * 128],
                                                     qs, start=True, stop=True)
                                    m = ti - NT_PAST - 4 * sc
                                    if m >= 0:
                                        nc.vector.tensor_add(
                                            sc_ps[:, hf * SCW:(hf + 1) * SCW],
                                            sc_ps[:, hf * SCW:(hf + 1) * SCW],
                                            mask_sb[:, m * SCW:(m + 1) * SCW])
                                ex = expp.tile([128, 2 * SCW], F32R)
                                nc.scalar.activation(ex[:], sc_ps[:], EXP, scale=SCALE)
                                for hf, ti in enumerate(pair):
                                    first = (pi == 0 and hf == 0)
                                    last = (pi == len(pairs) - 1 and hf == 1)
                                    nc.tensor.matmul(ctx_ps[:],
                                                     vatt[:, ti * 128:(ti + 1) * 128],
                                                     ex[:, hf * SCW:(hf + 1) * SCW],
                                                     start=first, stop=last)
                                    nc.tensor.matmul(sum_ps[:], ones_t[:],
                                                     ex[:, hf * SCW:(hf + 1) * SCW],
                                                     start=first, stop=last)
                            sum_sb = smallp.tile([1, SCW], F32R, tag="sums")
                            nc.vector.tensor_copy(sum_sb[:], sum_ps[:])
                            rep_ps = repps.tile([128, SCW], F32)
                            nc.tensor.matmul(rep_ps[:], ones_k[:], sum_sb[:],
                                             start=True, stop=True)
                            recip = smallp.tile([128, SCW], F32, tag="recip")
                            nc.vector.reciprocal(recip[:], rep_ps[:])
                            nc.vector.tensor_mul(ctxT[b][l][:, sc * SCW:(sc + 1) * SCW],
                                                 ctx_ps[:], recip[:])

            # ---- phase 3: O projection + reduce-scatter ----
            with (
                tc.tile_pool(name="wop", bufs=1) as wop,
                tc.tile_pool(name="ops", bufs=2, space="PSUM") as ops_,
                tc.tile_pool(name="osb", bufs=3) as osbp,
            ):
                wo = wop.tile([128, HL * E], F32R, tag="wo")  # [p, l*E + f]
                nc.sync.dma_start(
                    out=wo[:].rearrange("p (l f) -> p l f", l=HL),
                    in_=woT[:].rearrange("(l p) f -> p l f", p=128).bitcast(F32R))
                for b in range(B):
                    for mt in range(S // 128):
                        for fc in range(E // 512):
                            o_ps = ops_.tile([128, 512], F32)
                            for l in range(HL):
                                nc.tensor.matmul(o_ps[:],
                                                 ctxT[b][l][:, mt * 128:(mt + 1) * 128],
                                                 wo[:, l * E + fc * 512: l * E + (fc + 1) * 512],
                                                 start=(l == 0), stop=(l == HL - 1))
                            o_sb = osbp.tile([128, 512], F32)
                            nc.vector.tensor_add(o_sb[:], o_ps[:],
                                                 bo_sb[:, fc * 512:(fc + 1) * 512])
                            nc.sync.dma_start(
                                out=o_partial[b * S + mt * 128: b * S + (mt + 1) * 128,
                                              fc * 512:(fc + 1) * 512],
                                in_=o_sb[:])
                cc = nc.gpsimd.collective_compute(
                    "ReduceScatter", mybir.AluOpType.add,
                    replica_groups=[list(range(NCORES))],
                    ins=[o_partial[:]], outs=[rs_out[:]],
                )
                osd = nc.sync.dma_start(out=o_shard[:], in_=rs_out[:])
                add_dep_helper(osd.ins, cc.ins, sync=True, reason="rs_out RAW")

    _fix_walrus_wait_limit(nc)
    return nc


def _in_maps(x, past_k, past_v, Wq, bq, Wk, bk, Wv, bv, Wo, bo):
    f = np.float32
    xT = np.ascontiguousarray(x.reshape(T, E).T, dtype=f)
    masks = np.zeros((4, 128, SCW), f)
    p = np.arange(128)[:, None]
    fidx = np.arange(SCW)[None, :]
    for m in range(4):
        masks[m][(p + m * 128) > fidx] = NEG
    maps = []
    for c in range(NCORES):
        ce = slice(c * DL, (c + 1) * DL)
        maps.append({
            "xT": xT,
            "wqT": np.ascontiguousarray(Wq[ce, :].T, dtype=f),
            "wkT": np.ascontiguousarray(Wk[ce, :].T, dtype=f),
            "wvT": np.ascontiguousarray(Wv[ce, :].T, dtype=f),
            "woT": np.ascontiguousarray(Wo[:, ce].T, dtype=f),
            "bq2": np.ascontiguousarray(bq[ce].reshape(HL, 128), dtype=f),
            "bk2": np.ascontiguousarray(bk[ce].reshape(HL, 128), dtype=f),
            "bv_rep": np.ascontiguousarray(np.broadcast_to(bv[ce], (128, DL)), dtype=f),
            "bo_rep": np.ascontiguousarray(np.broadcast_to(bo / NCORES, (128, E)), dtype=f),
            "pkT": np.ascontiguousarray(
                past_k[:, c * HL:(c + 1) * HL].transpose(0, 1, 3, 2), dtype=f),
            "pv": np.ascontiguousarray(past_v[:, c * HL:(c + 1) * HL], dtype=f),
            "masks": masks,
            "ones128": np.ones(128, f),
        })
    return maps


def kernel(x, past_k, past_v, Wq, bq, Wk, bk, Wv, bv, Wo, bo):
    x = np.asarray(x, np.float32)
    past_k = np.asarray(past_k, np.float32)
    past_v = np.asarray(past_v, np.float32)
    args = [np.asarray(a, np.float32) for a in (Wq, bq, Wk, bk, Wv, bv, Wo, bo)]

    if "nc" not in _CACHE:
        _CACHE["nc"] = _build()
    nc = _CACHE["nc"]

    maps = _in_maps(x, past_k, past_v, *args)
    res = run_bass_kernel_spmd(nc, maps, list(range(NCORES))).results

    out = np.concatenate([res[c]["o_shard"] for c in range(NCORES)], axis=0)
    out = out.reshape(B, S, E)

    k_full = np.empty((B, H, KV, DH), np.float32)
    v_full = np.empty((B, H, KV, DH), np.float32)
    k_full[:, :, :PAST] = past_k
    v_full[:, :, :PAST] = past_v
    for c in range(NCORES):
        kt = res[c]["kT_new"].reshape(HL, DH, B, S)      # [l, d, b, s]
        k_full[:, c * HL:(c + 1) * HL, PAST:] = kt.transpose(2, 0, 3, 1)
        vn = res[c]["v_new"].reshape(B, S, HL, DH)       # [b, s, l, d]
        v_full[:, c * HL:(c + 1) * HL, PAST:] = vn.transpose(0, 2, 1, 3)
    return out, k_full, v_full


# revision 14
# speedup vs baseline: 1811.9551x; 1811.9551x over previous
"""Multi-head attention with KV cache, tensor-parallel over 8 TRN2 NeuronCores.

Sharding: heads are split 2-per-core (H=16, 8 cores). Each core computes its
heads' Q/K/V projections (output columns), attention, and a partial O
projection (input rows); a ReduceScatter sums the partials and hands each core
one token shard of the final output. The host reassembles full outputs.

All matmuls run as float32r (full-rate PE, ~1.5e-4 relative error).
"""

import numpy as np

import concourse.bass as bass
import concourse.mybir as mybir
from concourse.tile import TileContext
from concourse.bass_utils import run_bass_kernel_spmd
from concourse.tile_rust import add_dep_helper

F32 = mybir.dt.float32
F32R = mybir.dt.float32r
EXP = mybir.ActivationFunctionType.Exp

B, S, E = 2, 2048, 2048
H, DH, PAST = 16, 128, 2048
NCORES = 8
HL = H // NCORES          # heads per core (2)
DL = HL * DH              # per-core head dims (256)
T = B * S                 # total tokens (4096)
CH = 512                  # projection token-chunk
NCHUNK = T // CH          # 16
SCW = 512                 # attention query-chunk width
NSC = S // SCW            # 4 query chunks per batch
KV = PAST + S             # 4096
NT = KV // 128            # 32 key tiles per (batch, head)
NT_PAST = PAST // 128     # 16
NEG = -1.0e30
SCALE = float(1.0 / np.sqrt(np.float32(DH)))

_CACHE = {}


def _fix_walrus_wait_limit(nc, max_waits=1):
    """This walrus build rejects >1 sync-wait command per instruction; move
    excess waits onto same-engine nops inserted right before the offender."""
    end_bb = nc.cur_bb.bb
    for fn in nc.m.functions:
        for bb in fn.blocks:
            insts = bb.instructions
            i = 0
            while i < len(insts):
                inst = insts[i]
                si = inst.sync_info
                if si is None or not si.on_wait or len(si.on_wait) <= max_waits:
                    i += 1
                    continue
                waits = list(si.on_wait)
                keep = waits[-max_waits:]
                extra = waits[:-max_waits]
                si.on_wait.clear()
                si.on_wait.extend(keep)
                eng = nc.engines[inst.engine]
                nops = []
                for j in range(0, len(extra), max_waits):
                    nop_inst = eng.nop(nofuse=True, hint="waitsplit").ins
                    nsi = nop_inst.sync_info
                    if nsi is None:
                        nop_inst.sync_info = mybir.SyncInfo(on_wait=[], on_update=[])
                        nsi = nop_inst.sync_info
                    nsi.on_wait.extend(extra[j:j + max_waits])
                    end_bb.instructions.remove(nop_inst)
                    nops.append(nop_inst)
                for j, n in enumerate(nops):
                    insts.insert(i + j, n)
                i += len(nops) + 1


def _build():
    nc = bass.Bass(num_devices=NCORES)

    xT = nc.declare_dram_parameter("xT", [E, T], F32, isOutput=False)
    wqT = nc.declare_dram_parameter("wqT", [E, DL], F32, isOutput=False)
    wkT = nc.declare_dram_parameter("wkT", [E, DL], F32, isOutput=False)
    wvT = nc.declare_dram_parameter("wvT", [E, DL], F32, isOutput=False)
    woT = nc.declare_dram_parameter("woT", [DL, E], F32, isOutput=False)
    bq2 = nc.declare_dram_parameter("bq2", [HL, 128], F32, isOutput=False)
    bk2 = nc.declare_dram_parameter("bk2", [HL, 128], F32, isOutput=False)
    bv_rep = nc.declare_dram_parameter("bv_rep", [128, DL], F32, isOutput=False)
    bo_rep = nc.declare_dram_parameter("bo_rep", [128, E], F32, isOutput=False)
    pkT = nc.declare_dram_parameter("pkT", [B, HL, 128, PAST], F32, isOutput=False)
    pv = nc.declare_dram_parameter("pv", [B, HL, PAST, 128], F32, isOutput=False)
    masks = nc.declare_dram_parameter("masks", [4, 128, SCW], F32, isOutput=False)
    ones128 = nc.declare_dram_parameter("ones128", [128], F32, isOutput=False)

    kT_new = nc.declare_dram_parameter("kT_new", [HL, 128, T], F32, isOutput=True)
    v_new = nc.declare_dram_parameter("v_new", [T, DL], F32, isOutput=True)
    o_shard = nc.declare_dram_parameter("o_shard", [B, S // NCORES, E], F32, isOutput=True)

    o_part = [[nc.dram_tensor(f"o_part{b}{fh}", [S, E // 2], F32) for fh in range(2)]
              for b in range(B)]
    rs_out = [[nc.dram_tensor(f"rs_out{b}{fh}", [S // NCORES, E // 2], F32)
               for fh in range(2)] for b in range(B)]

    NKT = E // 128  # 16 contraction tiles for the projections
    kv_writers = {}  # (kind, b, l) or (kind, b) -> [instructions]

    with TileContext(nc) as tc:
        with (
            tc.tile_pool(name="const", bufs=1) as constp,
            tc.tile_pool(name="big", bufs=1) as bigp,
        ):
            # ---- small constants ----
            bq_sb = constp.tile([128, HL], F32, tag="bq")
            bk_sb = constp.tile([128, HL], F32, tag="bk")
            nc.sync.dma_start(out=bq_sb[:], in_=bq2[:].rearrange("l p -> p l"))
            nc.sync.dma_start(out=bk_sb[:], in_=bk2[:].rearrange("l p -> p l"))
            bv_sb = constp.tile([128, DL], F32, tag="bv")
            bo_sb = constp.tile([128, E], F32, tag="bo")
            nc.sync.dma_start(out=bv_sb[:], in_=bv_rep[:])
            nc.sync.dma_start(out=bo_sb[:], in_=bo_rep[:])
            mask_sb = constp.tile([128, 4 * SCW], F32, tag="mask")
            nc.sync.dma_start(out=mask_sb[:].rearrange("p (m f) -> p m f", m=4),
                              in_=masks[:].rearrange("m p f -> p m f"))
            ones_t = constp.tile([128, 1], F32R, tag="onest")
            ones_k = constp.tile([1, 128], F32R, tag="onesk")
            nc.sync.dma_start(out=ones_t[:], in_=ones128[:].unsqueeze(1).bitcast(F32R))
            nc.sync.dma_start(out=ones_k[:], in_=ones128[:].unsqueeze(0).bitcast(F32R))

            # ---- persistent activations (SBUF resident across phases) ----
            qT_sb = [bigp.tile([128, T], F32R, name=f"qT{l}", tag=f"qT{l}")
                     for l in range(HL)]
            ctxT = [[bigp.tile([128, S], F32R, name=f"cx{b}{l}", tag=f"cx{b}{l}")
                     for l in range(HL)] for b in range(B)]

            # ---- phase 1: projections ----
            with (
                tc.tile_pool(name="wp", bufs=1) as wp,
                tc.tile_pool(name="xt", bufs=2) as xtp,
                tc.tile_pool(name="pjps", bufs=2, space="PSUM") as pjps,
                tc.tile_pool(name="ev", bufs=3) as evp,
            ):
                wq = wp.tile([128, NKT * DL], F32R, tag="wq")  # [p, kt*DL + m]
                wk = wp.tile([128, NKT * DL], F32R, tag="wk")
                wv = wp.tile([128, NKT * DL], F32R, tag="wv")
                for wt, wsrc in ((wq, wqT), (wk, wkT), (wv, wvT)):
                    for wc in range(4):
                        kl, kh = wc * (NKT // 4), (wc + 1) * (NKT // 4)
                        nc.sync.dma_start(
                            out=wt[:, kl * DL:kh * DL].rearrange("p (a m) -> p a m", a=NKT // 4),
                            in_=wsrc[kl * 128:kh * 128, :]
                                .rearrange("(a p) m -> p a m", p=128).bitcast(F32R))
                for ci in range(NCHUNK):
                    t0 = ci * CH
                    b = t0 // S
                    xt = xtp.tile([128, NKT * CH], F32R)  # [p, kt*CH + t]
                    for xc in range(4):
                        kl, kh = xc * (NKT // 4), (xc + 1) * (NKT // 4)
                        nc.sync.dma_start(
                            out=xt[:, kl * CH:kh * CH].rearrange("p (a t) -> p a t", a=NKT // 4),
                            in_=xT[kl * 128:kh * 128, t0:t0 + CH]
                                .rearrange("(a p) t -> p a t", p=128).bitcast(F32R),
                        )
                    # q/k projections (transposed layout: [dh, tok])
                    for l in range(HL):
                        pqk = pjps.tile([128, 2 * CH], F32, tag="pqk")
                        # q group fully, then k group: a start=True clears the
                        # whole bank's has_written bits, so groups sharing a
                        # bank must not interleave
                        for kt in range(NKT):
                            st, sp = (kt == 0), (kt == NKT - 1)
                            nc.tensor.matmul(pqk[:, 0:CH],
                                             wq[:, kt * DL + l * 128: kt * DL + (l + 1) * 128],
                                             xt[:, kt * CH:(kt + 1) * CH], start=st, stop=sp)
                        for kt in range(NKT):
                            st, sp = (kt == 0), (kt == NKT - 1)
                            nc.tensor.matmul(pqk[:, CH:2 * CH],
                                             wk[:, kt * DL + l * 128: kt * DL + (l + 1) * 128],
                                             xt[:, kt * CH:(kt + 1) * CH], start=st, stop=sp)
                        nc.vector.tensor_scalar_add(qT_sb[l][:, t0:t0 + CH], pqk[:, 0:CH],
                                                    bq_sb[:, l:l + 1])
                        kstg = evp.tile([128, CH], F32R, tag="kstg")
                        nc.vector.tensor_scalar_add(kstg[:], pqk[:, CH:2 * CH],
                                                    bk_sb[:, l:l + 1])
                        kw = nc.sync.dma_start(out=kT_new[l, :, t0:t0 + CH],
                                               in_=kstg[:].bitcast(F32))
                        kv_writers.setdefault(("k", b, l), []).append(kw.ins)
                    # v projection (natural layout: [tok, dh])
                    pv_ps = pjps.tile([128, (CH // 128) * DL], F32, tag="pv")
                    for mt in range(CH // 128):
                        for kt in range(NKT):
                            nc.tensor.matmul(pv_ps[:, mt * DL:(mt + 1) * DL],
                                             xt[:, kt * CH + mt * 128: kt * CH + (mt + 1) * 128],
                                             wv[:, kt * DL:(kt + 1) * DL],
                                             start=(kt == 0), stop=(kt == NKT - 1))
                    for mt in range(CH // 128):
                        g = ci * (CH // 128) + mt          # global token tile
                        vf = evp.tile([128, DL], F32, tag="vf")
                        nc.vector.tensor_add(vf[:], pv_ps[:, mt * DL:(mt + 1) * DL], bv_sb[:])
                        vw = nc.sync.dma_start(out=v_new[g * 128:(g + 1) * 128, :], in_=vf[:])
                        kv_writers.setdefault(("v", b), []).append(vw.ins)

            # ---- phase 2+3: attention, O projection, chunked reduce-scatter ----
            # PSUM budget (8 banks): scps 2x[128,1024]=4, ctxps 1, sumps 1,
            # ops 2 (shared between sum-replication tiles and O-proj psum)
            with (
                tc.tile_pool(name="kv", bufs=2) as kvp,
                tc.tile_pool(name="scps", bufs=2, space="PSUM") as scps,
                tc.tile_pool(name="expp", bufs=3) as expp,
                tc.tile_pool(name="ctxps", bufs=1, space="PSUM") as ctxps,
                tc.tile_pool(name="ops", bufs=3, space="PSUM") as ops_,
                tc.tile_pool(name="small", bufs=2) as smallp,
                tc.tile_pool(name="wop", bufs=1) as wop,
                tc.tile_pool(name="osb", bufs=3) as osbp,
            ):
                wo = wop.tile([128, HL * E], F32R, tag="wo")  # [p, l*E + f]
                nc.sync.dma_start(
                    out=wo[:].rearrange("p (l f) -> p l f", l=HL),
                    in_=woT[:].rearrange("(l p) f -> p l f", p=128).bitcast(F32R))
                for b in range(B):
                    for l in range(HL):
                        # full K^T and V for this (batch, head): past ++ new
                        katt = kvp.tile([128, NT * 128], F32R, tag="katt")
                        for q4 in range(0, PAST, 1024):
                            nc.sync.dma_start(out=katt[:, q4:q4 + 1024],
                                              in_=pkT[b, l, :, q4:q4 + 1024].bitcast(F32R))
                        for q4 in range(0, S, 1024):
                            krd = nc.sync.dma_start(
                                out=katt[:, PAST + q4:PAST + q4 + 1024],
                                in_=kT_new[l, :, b * S + q4: b * S + q4 + 1024].bitcast(F32R))
                            for w in kv_writers.get(("k", b, l), []):
                                add_dep_helper(krd.ins, w, sync=True, reason="kT_new RAW")
                        vatt = kvp.tile([128, NT * 128], F32R, tag="vatt")
                        for q4 in range(0, PAST, 1024):
                            nc.sync.dma_start(
                                out=vatt[:, q4:q4 + 1024].rearrange("p (a d) -> p a d", a=8),
                                in_=pv[b, l, q4:q4 + 1024]
                                    .rearrange("(a p) d -> p a d", p=128).bitcast(F32R))
                        for q4 in range(0, S, 1024):
                            vrd = nc.sync.dma_start(
                                out=vatt[:, PAST + q4:PAST + q4 + 1024]
                                    .rearrange("p (a d) -> p a d", a=8),
                                in_=v_new[b * S + q4: b * S + q4 + 1024, l * DH:(l + 1) * DH]
                                    .rearrange("(a p) d -> p a d", p=128).bitcast(F32R))
                            for w in kv_writers.get(("v", b), []):
                                add_dep_helper(vrd.ins, w, sync=True, reason="v_new RAW")
                        for sc in range(NSC):
                            qs = qT_sb[l][:, b * S + sc * SCW: b * S + (sc + 1) * SCW]
                            ctx_ps = ctxps.tile([128, SCW], F32)
                            sum_ps = ops_.tile([1, SCW], F32, tag="ops", name="sum_ps")
                            n_vis = NT_PAST + 4 * sc + 4   # visible key tiles
                            pairs = [(i, i + 1) for i in range(0, n_vis, 2)]
                            for pi, pair in enumerate(pairs):
                                sc_ps = scps.tile([128, 2 * SCW], F32)
                                for hf, ti in enumerate(pair):
                                    nc.tensor.matmul(sc_ps[:, hf * SCW:(hf + 1) * SCW],
                                                     katt[:, ti * 128:(ti + 1) * 128],
                                                     qs, start=True, stop=True)
                                    m = ti - NT_PAST - 4 * sc
                                    if m >= 0:
                                        nc.vector.tensor_add(
                                            sc_ps[:, hf * SCW:(hf + 1) * SCW],
                                            sc_ps[:, hf * SCW:(hf + 1) * SCW],
                                            mask_sb[:, m * SCW:(m + 1) * SCW])
                                ex = expp.tile([128, 2 * SCW], F32R)
                                nc.scalar.activation(ex[:], sc_ps[:], EXP, scale=SCALE)
                                for hf, ti in enumerate(pair):
                                    first = (pi == 0 and hf == 0)
                                    last = (pi == len(pairs) - 1 and hf == 1)
                                    nc.tensor.matmul(ctx_ps[:],
                                                     vatt[:, ti * 128:(ti + 1) * 128],
                                                     ex[:, hf * SCW:(hf + 1) * SCW],
                                                     start=first, stop=last)
                                    nc.tensor.matmul(sum_ps[:], ones_t[:],
                                                     ex[:, hf * SCW:(hf + 1) * SCW],
                                                     start=first, stop=last)
                            sum_sb = smallp.tile([1, SCW], F32R, tag="sums")
                            nc.vector.tensor_copy(sum_sb[:], sum_ps[:])
                            rep_ps = ops_.tile([128, SCW], F32, tag="ops", name="rep_ps")
                            nc.tensor.matmul(rep_ps[:], ones_k[:], sum_sb[:],
                                             start=True, stop=True)
                            recip = smallp.tile([128, SCW], F32, tag="recip")
                            nc.vector.reciprocal(recip[:], rep_ps[:])
                            nc.vector.tensor_mul(ctxT[b][l][:, sc * SCW:(sc + 1) * SCW],
                                                 ctx_ps[:], recip[:])
                    # O projection for this batch, in column halves, each
                    # reduce-scattered while later compute proceeds
                    EH = E // 2
                    for fh in range(2):
                        for mt in range(S // 128):
                            for f2 in range(EH // 512):
                                fc = fh * (EH // 512) + f2
                                o_ps = ops_.tile([128, 512], F32, tag="ops", name="o_ps")
                                for l in range(HL):
                                    nc.tensor.matmul(o_ps[:],
                                                     ctxT[b][l][:, mt * 128:(mt + 1) * 128],
                                                     wo[:, l * E + fc * 512: l * E + (fc + 1) * 512],
                                                     start=(l == 0), stop=(l == HL - 1))
                                o_sb = osbp.tile([128, 512], F32)
                                nc.vector.tensor_add(o_sb[:], o_ps[:],
                                                     bo_sb[:, fc * 512:(fc + 1) * 512])
                                nc.sync.dma_start(
                                    out=o_part[b][fh][mt * 128:(mt + 1) * 128,
                                                      f2 * 512:(f2 + 1) * 512],
                                    in_=o_sb[:])
                        cc = nc.gpsimd.collective_compute(
                            "ReduceScatter", mybir.AluOpType.add,
                            replica_groups=[list(range(NCORES))],
                            ins=[o_part[b][fh][:]], outs=[rs_out[b][fh][:]],
                        )
                        osd = nc.sync.dma_start(
                            out=o_shard[b, :, fh * EH:(fh + 1) * EH],
                            in_=rs_out[b][fh][:])
                        add_dep_helper(osd.ins, cc.ins, sync=True, reason="rs_out RAW")

    _fix_walrus_wait_limit(nc)
    return nc


def _in_maps(x, past_k, past_v, Wq, bq, Wk, bk, Wv, bv, Wo, bo):
    f = np.float32
    xT = np.ascontiguousarray(x.reshape(T, E).T, dtype=f)
    masks = np.zeros((4, 128, SCW), f)
    p = np.arange(128)[:, None]
    fidx = np.arange(SCW)[None, :]
    for m in range(4):
        masks[m][(p + m * 128) > fidx] = NEG
    maps = []
    for c in range(NCORES):
        ce = slice(c * DL, (c + 1) * DL)
        maps.append({
            "xT": xT,
            "wqT": np.ascontiguousarray(Wq[ce, :].T, dtype=f),
            "wkT": np.ascontiguousarray(Wk[ce, :].T, dtype=f),
            "wvT": np.ascontiguousarray(Wv[ce, :].T, dtype=f),
            "woT": np.ascontiguousarray(Wo[:, ce].T, dtype=f),
            "bq2": np.ascontiguousarray(bq[ce].reshape(HL, 128), dtype=f),
            "bk2": np.ascontiguousarray(bk[ce].reshape(HL, 128), dtype=f),
            "bv_rep": np.ascontiguousarray(np.broadcast_to(bv[ce], (128, DL)), dtype=f),
            "bo_rep": np.ascontiguousarray(np.broadcast_to(bo / NCORES, (128, E)), dtype=f),
            "pkT": np.ascontiguousarray(
                past_k[:, c * HL:(c + 1) * HL].transpose(0, 1, 3, 2), dtype=f),
            "pv": np.ascontiguousarray(past_v[:, c * HL:(c + 1) * HL], dtype=f),
            "masks": masks,
            "ones128": np.ones(128, f),
        })
    return maps


def kernel(x, past_k, past_v, Wq, bq, Wk, bk, Wv, bv, Wo, bo):
    x = np.asarray(x, np.float32)
    past_k = np.asarray(past_k, np.float32)
    past_v = np.asarray(past_v, np.float32)
    args = [np.asarray(a, np.float32) for a in (Wq, bq, Wk, bk, Wv, bv, Wo, bo)]

    if "nc" not in _CACHE:
        _CACHE["nc"] = _build()
    nc = _CACHE["nc"]

    maps = _in_maps(x, past_k, past_v, *args)
    res = run_bass_kernel_spmd(nc, maps, list(range(NCORES))).results

    out = np.empty((B, S, E), np.float32)
    for c in range(NCORES):
        sh = res[c]["o_shard"]            # [B, S//NCORES, E]
        for b in range(B):
            out[b, c * (S // NCORES):(c + 1) * (S // NCORES)] = sh[b]

    k_full = np.empty((B, H, KV, DH), np.float32)
    v_full = np.empty((B, H, KV, DH), np.float32)
    k_full[:, :, :PAST] = past_k
    v_full[:, :, :PAST] = past_v
    for c in range(NCORES):
        kt = res[c]["kT_new"].reshape(HL, DH, B, S)      # [l, d, b, s]
        k_full[:, c * HL:(c + 1) * HL, PAST:] = kt.transpose(2, 0, 3, 1)
        vn = res[c]["v_new"].reshape(B, S, HL, DH)       # [b, s, l, d]
        v_full[:, c * HL:(c + 1) * HL, PAST:] = vn.transpose(0, 2, 1, 3)
    return out, k_full, v_full
